# revision 1
# baseline (speedup 1.0000x reference)
"""RGCN-BDD link-predict layer kernel for 8 TRN2 NeuronCores.

Strategy: shard edges by destination-node slice (6250 nodes/device) so the
segment-sum is fully local; run the two RGCN layers as two launches of one
compiled single-layer NEFF, with host-side ReLU/bias between launches.

Per device, per layer:
  stage 1: per 128-edge tile, indirect-gather src features and per-edge
           block-diagonal weight rows; messages via strided DVE ops
           (block-diag einsum); norm scaling on the scalar engine.
  stage 2: edges are dst-sorted, so each 128-node chunk's messages are a
           contiguous row range of the message buffer; segment-sum is done
           on the tensor engine with host-built one-hot matrices, fused
           with the self-loop matmul in one PSUM accumulation group.
"""
import sys
if '/opt/trn_rl_repo' not in sys.path:
    sys.path.insert(0, '/opt/trn_rl_repo')

import numpy as np

import concourse.bass as bass
import concourse.bacc as bacc
import concourse.mybir as mybir
import concourse.tile as tile
from concourse.bass_utils import run_bass_kernel_spmd

# problem constants (hardcoded per spec)
NN = 50000      # num nodes
H = 500         # hidden dim
NB = 100        # num bases
SUB = 5         # block size
W_COLS = NB * SUB * SUB  # 2500
NR2 = 474       # num relations * 2
E = 100000      # num edges
NDEV = 8
P = 128
NPD = NN // NDEV          # 6250 nodes per device
NCH = (NPD + P - 1) // P  # 49 chunks
N_PAD = NCH * P           # 6272
KQ = [128, 128, 128, H - 3 * 128]  # K-chunks of the 500-dim contraction

_cache = {}


def _plan(src, dst, etype, norm):
    """Host-side sharding plan; layer-invariant."""
    src = np.asarray(src).astype(np.int64)
    dst = np.asarray(dst).astype(np.int64)
    etype = np.asarray(etype).astype(np.int64)
    norm = np.asarray(norm).astype(np.float32).reshape(-1)

    dev_of = dst // NPD
    per = []
    for d in range(NDEV):
        sel = np.nonzero(dev_of == d)[0]
        dl = dst[sel] - d * NPD
        order = np.argsort(dl, kind='stable')
        el = sel[order]
        per.append((el, dl[order]))
    n_max = max(len(el) for el, _ in per)
    ET = (n_max + P - 1) // P
    E_PAD = ET * P

    # unique-src compaction (reduces per-device upload of the gather table)
    uniqs, srcn_cols = [], []
    for el, _ in per:
        u, inv = np.unique(src[el], return_inverse=True)
        uniqs.append(u)
        srcn_cols.append(inv)
    U_PAD = max(len(u) for u in uniqs)

    # per-chunk union windows over edge tiles
    W0 = np.zeros(NCH, np.int64)
    WEND = np.zeros(NCH, np.int64)
    for c in range(NCH):
        lo, hi = [], []
        for el, dl in per:
            e0 = np.searchsorted(dl, c * P, 'left')
            e1 = np.searchsorted(dl, (c + 1) * P, 'left')
            lo.append(e0 // P)
            hi.append((e1 + P - 1) // P if e1 > 0 else 0)
        W0[c] = min(lo)
        WEND[c] = max(max(hi), W0[c] + 1)
    WEND = np.minimum(WEND, ET)
    W0 = np.minimum(W0, WEND - 1)
    KE = (WEND - W0).astype(np.int64)
    OHT = int(KE.sum())           # total one-hot tiles
    ohoff = np.concatenate([[0], np.cumsum(KE)])[:NCH].astype(np.int64)

    # per-device static input arrays
    srcn = np.zeros((NDEV, P, ET), np.int32)
    etn = np.zeros((NDEV, P, ET), np.int32)
    nrm = np.zeros((NDEV, P, ET), np.float32)
    oh = np.zeros((NDEV, OHT * P, P), np.float32)
    gidx = []   # per device: el (edge ids in MSG order)
    for d in range(NDEV):
        el, dl = per[d]
        n_d = len(el)
        pad = E_PAD - n_d
        col = np.pad(srcn_cols[d], (0, pad)).astype(np.int32)
        srcn[d] = col.reshape(ET, P).T
        etn[d] = np.pad(etype[el], (0, pad)).astype(np.int32).reshape(ET, P).T
        nrm[d] = np.pad(norm[el], (0, pad)).astype(np.float32).reshape(ET, P).T
        # one-hots
        for c in range(NCH):
            for kk in range(KE[c]):
                g0 = (W0[c] + kk) * P
                rows = np.arange(g0, g0 + P)
                valid = rows < n_d
                m = dl[rows[valid]] - c * P
                ok = (m >= 0) & (m < P)
                j = np.nonzero(valid)[0][ok]
                oh[d, (ohoff[c] + kk) * P + j, m[ok]] = 1.0
        gidx.append(el)

    return dict(per=per, ET=ET, E_PAD=E_PAD, U_PAD=U_PAD, uniqs=uniqs,
                srcn=srcn, etn=etn, nrm=nrm, oh=oh, W0=W0, KE=KE,
                ohoff=ohoff, OHT=OHT, gidx=gidx)


def _build_nc(ET, U_PAD, W0, KE, ohoff, OHT):
    nc = bacc.Bacc(None, target_bir_lowering=False)
    f32, i32 = mybir.dt.float32, mybir.dt.int32

    xg = nc.dram_tensor("xg", [U_PAD, H], f32, kind="ExternalInput")
    xtp = nc.dram_tensor("xtp", [H, N_PAD], f32, kind="ExternalInput")
    wf = nc.dram_tensor("wf", [NR2, W_COLS], f32, kind="ExternalInput")
    lw = nc.dram_tensor("lw", [H, H], f32, kind="ExternalInput")
    srcn = nc.dram_tensor("srcn", [P, ET], i32, kind="ExternalInput")
    etn = nc.dram_tensor("etn", [P, ET], i32, kind="ExternalInput")
    nrm = nc.dram_tensor("nrm", [P, ET], f32, kind="ExternalInput")
    oh = nc.dram_tensor("oh", [OHT * P, P], f32, kind="ExternalInput")
    out = nc.dram_tensor("out", [N_PAD, H], f32, kind="ExternalOutput")

    with tile.TileContext(nc) as tc:
        with tc.tile_pool(name="const", bufs=1) as constp, \
             tc.tile_pool(name="s1", bufs=3) as s1, \
             tc.tile_pool(name="wgp", bufs=3) as wgp, \
             tc.tile_pool(name="s2", bufs=4) as s2, \
             tc.tile_pool(name="psum", bufs=4, space="PSUM") as psp, \
             tc.tile_pool(name="dram", bufs=1, space="DRAM") as dram:

            MSG = dram.tile([ET * P, H], f32)

            # preload loop weights (rhs tiles, K on partitions) and indices
            lw_sb = []
            for q in range(4):
                t = constp.tile([P, H], f32, tag=f"lw{q}")
                nc.sync.dma_start(out=t[:KQ[q], :], in_=lw[q * 128:q * 128 + KQ[q], :])
                lw_sb.append(t)
            srcn_sb = constp.tile([P, ET], i32, tag="srcn")
            etn_sb = constp.tile([P, ET], i32, tag="etn")
            nrm_sb = constp.tile([P, ET], f32, tag="nrm")
            nc.sync.dma_start(out=srcn_sb[:], in_=srcn[:, :])
            nc.sync.dma_start(out=etn_sb[:], in_=etn[:, :])
            nc.sync.dma_start(out=nrm_sb[:], in_=nrm[:, :])

            # ---- stage 1: messages ----
            for t in range(ET):
                xe = s1.tile([P, H], f32, tag="xe")
                wg = wgp.tile([P, W_COLS], f32, tag="wg")
                nc.gpsimd.indirect_dma_start(
                    out=xe[:], out_offset=None, in_=xg[:, :],
                    in_offset=bass.IndirectOffsetOnAxis(ap=srcn_sb[:, t:t + 1], axis=0))
                nc.gpsimd.indirect_dma_start(
                    out=wg[:], out_offset=None, in_=wf[:, :],
                    in_offset=bass.IndirectOffsetOnAxis(ap=etn_sb[:, t:t + 1], axis=0))
                xen = s1.tile([P, H], f32, tag="xen")
                nc.scalar.activation(out=xen[:], in_=xe[:],
                                     func=mybir.ActivationFunctionType.Copy,
                                     scale=nrm_sb[:, t:t + 1])
                msg = s1.tile([P, H], f32, tag="msg")
                tmp = s1.tile([P, H], f32, tag="tmp")
                xen_r = xen[:].rearrange("p (b i) -> p b i", i=SUB)
                wg_r = wg[:].rearrange("p (b i j) -> p b i j", i=SUB, j=SUB)
                msg_r = msg[:].rearrange("p (b j) -> p b j", j=SUB)
                tmp_r = tmp[:].rearrange("p (b j) -> p b j", j=SUB)
                for i in range(SUB):
                    xi = xen_r[:, :, i:i + 1].to_broadcast([P, NB, SUB])
                    dstv = msg_r if i == 0 else tmp_r
                    nc.vector.tensor_tensor(out=dstv, in0=xi, in1=wg_r[:, :, i, :],
                                            op=mybir.AluOpType.mult)
                    if i > 0:
                        nc.vector.tensor_tensor(out=msg_r, in0=msg_r, in1=tmp_r,
                                                op=mybir.AluOpType.add)
                nc.sync.dma_start(out=MSG[t * P:(t + 1) * P, :], in_=msg[:])

            # ---- stage 2: segment-sum (one-hot matmul) + self-loop ----
            for c in range(NCH):
                ps = psp.tile([P, H], f32, tag="ps")
                ke = int(KE[c])
                for kk in range(ke):
                    mb = s2.tile([P, H], f32, tag="mb")
                    ohp = s2.tile([P, P], f32, tag="ohp")
                    r0 = (int(W0[c]) + kk) * P
                    o0 = (int(ohoff[c]) + kk) * P
                    nc.sync.dma_start(out=mb[:], in_=MSG[r0:r0 + P, :])
                    nc.sync.dma_start(out=ohp[:], in_=oh[o0:o0 + P, :])
                    nc.tensor.matmul(out=ps[:], lhsT=ohp[:], rhs=mb[:],
                                     start=(kk == 0), stop=False)
                for q in range(4):
                    xt = s2.tile([P, P], f32, tag="xt")
                    kq = KQ[q]
                    nc.sync.dma_start(
                        out=xt[:kq, :],
                        in_=xtp[q * 128:q * 128 + kq, c * P:(c + 1) * P])
                    nc.tensor.matmul(out=ps[:], lhsT=xt[:kq, :],
                                     rhs=lw_sb[q][:kq, :],
                                     start=False, stop=(q == 3))
                outt = s2.tile([P, H], f32, tag="outt")
                nc.scalar.activation(out=outt[:], in_=ps[:],
                                     func=mybir.ActivationFunctionType.Copy)
                nc.sync.dma_start(out=out[c * P:(c + 1) * P, :], in_=outt[:])
    nc.finalize()
    return nc


def _run_layer(nc, plan, x, wfull, lwfull, trace=False):
    """One RGCN-BDD layer (pre-bias, pre-activation) on 8 cores."""
    x = np.ascontiguousarray(x, dtype=np.float32)
    wf = np.ascontiguousarray(wfull.reshape(NR2, W_COLS), dtype=np.float32)
    lw = np.ascontiguousarray(lwfull, dtype=np.float32)
    in_maps = []
    for d in range(NDEV):
        u = plan['uniqs'][d]
        xgd = np.zeros((plan['U_PAD'], H), np.float32)
        xgd[:len(u)] = x[u]
        xtpd = np.zeros((H, N_PAD), np.float32)
        xtpd[:, :NPD] = x[d * NPD:(d + 1) * NPD].T
        in_maps.append({
            "xg": xgd, "xtp": np.ascontiguousarray(xtpd), "wf": wf, "lw": lw,
            "srcn": plan['srcn'][d], "etn": plan['etn'][d],
            "nrm": plan['nrm'][d], "oh": plan['oh'][d],
        })
    res = run_bass_kernel_spmd(nc, in_maps, core_ids=list(range(NDEV)),
                               trace=trace)
    outp = np.empty((NN, H), np.float32)
    for d in range(NDEV):
        outp[d * NPD:(d + 1) * NPD] = res.results[d]["out"][:NPD]
    return outp, res


def kernel(nids, src, dst, etype, norm, emb, W1, loop_w1, bias1,
           W2, loop_w2, bias2, _trace=False, _times=None):
    key = "nc"
    if key not in _cache:
        plan = _plan(src, dst, etype, norm)
        nc = _build_nc(plan['ET'], plan['U_PAD'], plan['W0'], plan['KE'],
                       plan['ohoff'], plan['OHT'])
        _cache[key] = (plan, nc)
    plan, nc = _cache[key]

    x = np.asarray(emb, dtype=np.float32)[np.asarray(nids, dtype=np.int64)]
    h_pre, r1 = _run_layer(nc, plan, x, np.asarray(W1), np.asarray(loop_w1),
                           trace=_trace)
    h = np.maximum(h_pre + np.asarray(bias1, dtype=np.float32)[None, :], 0.0)
    out_pre, r2 = _run_layer(nc, plan, h, np.asarray(W2), np.asarray(loop_w2),
                             trace=_trace)
    out = out_pre + np.asarray(bias2, dtype=np.float32)[None, :]
    if _times is not None:
        _times.extend([r1, r2])
    return out


# revision 3
# speedup vs baseline: 2.2614x; 2.2614x over previous
"""RGCN-BDD link-predict layer kernel for 8 TRN2 NeuronCores.

Strategy: shard edges by destination-node slice (6250 nodes/device) so the
segment-sum is fully local; run the two RGCN layers as two launches of one
compiled single-layer NEFF, with host-side ReLU/bias between launches.

Per device, per layer (fused single pass, bf16 data / f32 accumulate):
  - edges are dst-sorted; per 128-node chunk the relevant edge tiles form a
    monotone sliding window, so per-edge product tiles stay SBUF-resident
    (no message roundtrip through DRAM).
  - per 128-edge tile: indirect-gather src features (xe) and per-edge
    block-diagonal weight rows (wg, host-permuted to [i, b, j] layout);
    the scalar engine expands xe to the [i, b, j] broadcast layout; one
    full-width DVE multiply forms all 2500 partial products; DVE pairwise
    adds fold some i-slices.
  - per chunk: segment-sum via tensor-engine matmuls with host-built
    one-hot matrices (entries carry the edge norm), accumulated in PSUM
    together with the remaining product i-slices and the self-loop matmul
    (x^T slices against the loop weight).
"""
import sys
if '/opt/trn_rl_repo' not in sys.path:
    sys.path.insert(0, '/opt/trn_rl_repo')

import numpy as np
import ml_dtypes

import concourse.bass as bass
import concourse.bacc as bacc
import concourse.mybir as mybir
import concourse.tile as tile
from concourse.bass_utils import run_bass_kernel_spmd

# problem constants (hardcoded per spec)
NN = 50000      # num nodes
H = 500         # hidden dim
NB = 100        # num bases
SUB = 5         # block size
W_COLS = NB * SUB * SUB  # 2500
NR2 = 474       # num relations * 2
E = 100000      # num edges
NDEV = 8
P = 128
NPD = NN // NDEV          # 6250 nodes per device
NCH = (NPD + P - 1) // P  # 49 chunks
N_PAD = NCH * P           # 6272
KQ = [128, 128, 128, H - 3 * 128]  # K-chunks of the 500-dim contraction
NADD = 2   # i-slice pairwise adds done on DVE (0..4); PE does 5-NADD matmuls

BF = mybir.dt.bfloat16
F32 = mybir.dt.float32
I32 = mybir.dt.int32

_cache = {}


def _plan(src, dst, etype, norm):
    """Host-side sharding plan; layer-invariant."""
    src = np.asarray(src).astype(np.int64)
    dst = np.asarray(dst).astype(np.int64)
    etype = np.asarray(etype).astype(np.int64)
    norm = np.asarray(norm).astype(np.float32).reshape(-1)

    dev_of = dst // NPD
    per = []
    for d in range(NDEV):
        sel = np.nonzero(dev_of == d)[0]
        dl = dst[sel] - d * NPD
        order = np.argsort(dl, kind='stable')
        el = sel[order]
        per.append((el, dl[order]))
    n_max = max(len(el) for el, _ in per)
    ET = (n_max + P - 1) // P

    # unique-src compaction (reduces per-device upload of the gather table)
    uniqs, srcn_cols = [], []
    for el, _ in per:
        u, inv = np.unique(src[el], return_inverse=True)
        uniqs.append(u)
        srcn_cols.append(inv)
    U_PAD = max(len(u) for u in uniqs)

    # per-chunk union windows over edge tiles (same for all devices)
    W0 = np.zeros(NCH, np.int64)
    WEND = np.zeros(NCH, np.int64)
    for c in range(NCH):
        lo, hi = [], []
        for el, dl in per:
            e0 = np.searchsorted(dl, c * P, 'left')
            e1 = np.searchsorted(dl, (c + 1) * P, 'left')
            lo.append(e0 // P)
            hi.append((e1 + P - 1) // P if e1 > 0 else 0)
        W0[c] = min(lo)
        WEND[c] = max(max(hi), W0[c] + 1)
    WEND = np.minimum(WEND, ET)
    W0 = np.minimum(W0, WEND - 1)
    KE = (WEND - W0).astype(np.int64)
    OHT = int(KE.sum())           # total one-hot tiles
    ohoff = np.concatenate([[0], np.cumsum(KE)])[:NCH].astype(np.int64)

    # per-device static input arrays
    srcn = np.zeros((NDEV, P, ET), np.int32)
    etn = np.zeros((NDEV, P, ET), np.int32)
    oh = np.zeros((NDEV, OHT * P, P), np.float32)
    for d in range(NDEV):
        el, dl = per[d]
        n_d = len(el)
        pad = ET * P - n_d
        col = np.pad(srcn_cols[d], (0, pad)).astype(np.int32)
        srcn[d] = col.reshape(ET, P).T
        etn[d] = np.pad(etype[el], (0, pad)).astype(np.int32).reshape(ET, P).T
        nr = norm[el]
        for c in range(NCH):
            for kk in range(KE[c]):
                g0 = (W0[c] + kk) * P
                rows = np.arange(g0, g0 + P)
                valid = rows < n_d
                m = dl[rows[valid]] - c * P
                ok = (m >= 0) & (m < P)
                j = np.nonzero(valid)[0][ok]
                oh[d, (ohoff[c] + kk) * P + j, m[ok]] = nr[rows[valid]][ok]

    return dict(ET=ET, U_PAD=U_PAD, uniqs=uniqs, srcn=srcn, etn=etn,
                oh=oh.astype(ml_dtypes.bfloat16), W0=W0, KE=KE, ohoff=ohoff,
                OHT=OHT)


def _build_nc(ET, U_PAD, W0, KE, ohoff, OHT):
    nc = bacc.Bacc(None, target_bir_lowering=False)

    xg = nc.dram_tensor("xg", [U_PAD, H], BF, kind="ExternalInput")
    xtp = nc.dram_tensor("xtp", [H, N_PAD], BF, kind="ExternalInput")
    wf = nc.dram_tensor("wf", [NR2, W_COLS], BF, kind="ExternalInput")
    lw = nc.dram_tensor("lw", [H, H], BF, kind="ExternalInput")
    srcn = nc.dram_tensor("srcn", [P, ET], I32, kind="ExternalInput")
    etn = nc.dram_tensor("etn", [P, ET], I32, kind="ExternalInput")
    oh = nc.dram_tensor("oh", [OHT * P, P], BF, kind="ExternalInput")
    out = nc.dram_tensor("out", [N_PAD, H], F32, kind="ExternalOutput")

    NMM = SUB - NADD  # product slices fed to PE per window tile

    with tile.TileContext(nc) as tc:
        with tc.tile_pool(name="const", bufs=1) as constp, \
             tc.tile_pool(name="s1", bufs=3) as s1, \
             tc.tile_pool(name="prodp", bufs=10) as prodp, \
             tc.tile_pool(name="s2", bufs=4) as s2, \
             tc.tile_pool(name="psum", bufs=4, space="PSUM") as psp:

            # preload loop weights (rhs tiles, K on partitions) and indices
            lw_sb = []
            for q in range(4):
                t = constp.tile([P, H], BF, tag=f"lw{q}")
                nc.sync.dma_start(out=t[:KQ[q], :], in_=lw[q * 128:q * 128 + KQ[q], :])
                lw_sb.append(t)
            srcn_sb = constp.tile([P, ET], I32, tag="srcn")
            etn_sb = constp.tile([P, ET], I32, tag="etn")
            nc.sync.dma_start(out=srcn_sb[:], in_=srcn[:, :])
            nc.sync.dma_start(out=etn_sb[:], in_=etn[:, :])

            prods = {}   # edge-tile idx -> list of NMM rhs views (+ backing tiles)

            def produce(t):
                xe = s1.tile([P, H], BF, tag="xe")
                wg = s1.tile([P, W_COLS], BF, tag="wg")
                nc.gpsimd.indirect_dma_start(
                    out=xe[:], out_offset=None, in_=xg[:, :],
                    in_offset=bass.IndirectOffsetOnAxis(ap=srcn_sb[:, t:t + 1], axis=0))
                nc.gpsimd.indirect_dma_start(
                    out=wg[:], out_offset=None, in_=wf[:, :],
                    in_offset=bass.IndirectOffsetOnAxis(ap=etn_sb[:, t:t + 1], axis=0))
                # expand xe[b*5+i] to [i, b, j] layout (broadcast over j) on ACT
                xex = s1.tile([P, W_COLS], BF, tag="xex")
                xe_v = xe[:].rearrange("p (b i) -> p i b", i=SUB)  # strided view
                nc.scalar.activation(
                    out=xex[:].rearrange("p (i b j) -> p i b j", i=SUB, j=SUB),
                    in_=xe_v.to_broadcast([P, SUB, NB, SUB]),
                    func=mybir.ActivationFunctionType.Copy)
                # one full-width multiply: all 2500 partial products
                prod = prodp.tile([P, W_COLS], BF, tag="prod")
                nc.vector.tensor_tensor(out=prod[:], in0=xex[:], in1=wg[:],
                                        op=mybir.AluOpType.mult)
                # fold NADD i-slices pairwise on DVE
                sl = [prod[:, i * H:(i + 1) * H] for i in range(SUB)]
                if NADD >= 1:
                    s01 = prodp.tile([P, H], BF, tag="s01")
                    nc.vector.tensor_tensor(out=s01[:], in0=sl[0], in1=sl[1],
                                            op=mybir.AluOpType.add)
                    sl = [s01[:]] + sl[2:]
                if NADD >= 2:
                    s23 = prodp.tile([P, H], BF, tag="s23")
                    nc.vector.tensor_tensor(out=s23[:], in0=sl[1], in1=sl[2],
                                            op=mybir.AluOpType.add)
                    sl = [sl[0], s23[:]] + sl[3:]
                if NADD >= 3:
                    s03 = prodp.tile([P, H], BF, tag="s03")
                    nc.vector.tensor_tensor(out=s03[:], in0=sl[0], in1=sl[1],
                                            op=mybir.AluOpType.add)
                    sl = [s03[:]] + sl[2:]
                if NADD >= 4:
                    s04 = prodp.tile([P, H], BF, tag="s04")
                    nc.vector.tensor_tensor(out=s04[:], in0=sl[0], in1=sl[1],
                                            op=mybir.AluOpType.add)
                    sl = [s04[:]] + sl[2:]
                assert len(sl) == NMM
                prods[t] = sl

            produced = 0
            for c in range(NCH):
                need = int(W0[c] + KE[c])
                while produced < need:
                    produce(produced)
                    produced += 1
                ps = psp.tile([P, H], F32, tag="ps")
                first = True
                for kk in range(int(KE[c])):
                    t = int(W0[c]) + kk
                    ohp = s2.tile([P, P], BF, tag="ohp")
                    o0 = (int(ohoff[c]) + kk) * P
                    nc.sync.dma_start(out=ohp[:], in_=oh[o0:o0 + P, :])
                    for rv in prods[t]:
                        nc.tensor.matmul(out=ps[:], lhsT=ohp[:], rhs=rv,
                                         start=first, stop=False)
                        first = False
                for q in range(4):
                    xt = s2.tile([P, P], BF, tag="xt")
                    kq = KQ[q]
                    nc.sync.dma_start(
                        out=xt[:kq, :],
                        in_=xtp[q * 128:q * 128 + kq, c * P:(c + 1) * P])
                    nc.tensor.matmul(out=ps[:], lhsT=xt[:kq, :],
                                     rhs=lw_sb[q][:kq, :],
                                     start=False, stop=(q == 3))
                outt = s2.tile([P, H], F32, tag="outt")
                nc.scalar.activation(out=outt[:], in_=ps[:],
                                     func=mybir.ActivationFunctionType.Copy)
                nc.sync.dma_start(out=out[c * P:(c + 1) * P, :], in_=outt[:])
                # drop window tiles no longer needed
                if c + 1 < NCH:
                    for t in [k for k in prods if k < int(W0[c + 1])]:
                        del prods[t]
    nc.finalize()
    return nc


def _run_layer(nc, plan, x, wfp, lwb, trace=False):
    """One RGCN-BDD layer (pre-bias, pre-activation) on 8 cores."""
    xb = x.astype(ml_dtypes.bfloat16)
    in_maps = []
    for d in range(NDEV):
        u = plan['uniqs'][d]
        xgd = np.zeros((plan['U_PAD'], H), ml_dtypes.bfloat16)
        xgd[:len(u)] = xb[u]
        xtpd = np.zeros((H, N_PAD), ml_dtypes.bfloat16)
        xtpd[:, :NPD] = xb[d * NPD:(d + 1) * NPD].T
        in_maps.append({
            "xg": xgd, "xtp": np.ascontiguousarray(xtpd), "wf": wfp, "lw": lwb,
            "srcn": plan['srcn'][d], "etn": plan['etn'][d], "oh": plan['oh'][d],
        })
    res = run_bass_kernel_spmd(nc, in_maps, core_ids=list(range(NDEV)),
                               trace=trace)
    outp = np.empty((NN, H), np.float32)
    for d in range(NDEV):
        outp[d * NPD:(d + 1) * NPD] = res.results[d]["out"][:NPD]
    return outp, res


def _permute_w(W):
    # [r, b, i, j] -> [r, i, b, j] flattened, bf16
    W = np.asarray(W, dtype=np.float32).reshape(NR2, NB, SUB, SUB)
    return np.ascontiguousarray(
        W.transpose(0, 2, 1, 3).reshape(NR2, W_COLS)).astype(ml_dtypes.bfloat16)


def kernel(nids, src, dst, etype, norm, emb, W1, loop_w1, bias1,
           W2, loop_w2, bias2, _trace=False, _times=None):
    key = "nc"
    if key not in _cache:
        plan = _plan(src, dst, etype, norm)
        nc = _build_nc(plan['ET'], plan['U_PAD'], plan['W0'], plan['KE'],
                       plan['ohoff'], plan['OHT'])
        _cache[key] = (plan, nc)
    plan, nc = _cache[key]

    x = np.asarray(emb, dtype=np.float32)[np.asarray(nids, dtype=np.int64)]
    h_pre, r1 = _run_layer(nc, plan, x, _permute_w(W1),
                           np.asarray(loop_w1, np.float32).astype(ml_dtypes.bfloat16),
                           trace=_trace)
    h = np.maximum(h_pre + np.asarray(bias1, dtype=np.float32)[None, :], 0.0)
    out_pre, r2 = _run_layer(nc, plan, h, _permute_w(W2),
                             np.asarray(loop_w2, np.float32).astype(ml_dtypes.bfloat16),
                             trace=_trace)
    out = out_pre + np.asarray(bias2, dtype=np.float32)[None, :]
    if _times is not None:
        _times.extend([r1, r2])
    return out


# revision 4
# speedup vs baseline: 2.8120x; 1.2435x over previous
"""RGCN-BDD link-predict layer kernel for 8 TRN2 NeuronCores.

Strategy: shard edges by destination-node slice (6250 nodes/device) so the
segment-sum is fully local; run the two RGCN layers as two launches of one
compiled single-layer NEFF, with host-side ReLU/bias between launches.

Per device, per layer (fused single pass, bf16 data / f32 accumulate):
  - edges are dst-sorted; per 128-node chunk the relevant edge tiles form a
    monotone sliding window, so per-edge product tiles stay SBUF-resident
    (no message roundtrip through DRAM).
  - per 128-edge tile: indirect-gather src features (xe) and per-edge
    block-diagonal weight rows (wg, host-permuted to [i, b, j] layout);
    the scalar engine expands xe to the [i, b, j] broadcast layout; one
    full-width DVE multiply forms all 2500 partial products; DVE pairwise
    adds fold some i-slices.
  - per chunk: segment-sum via tensor-engine matmuls with host-built
    one-hot matrices (entries carry the edge norm), accumulated in PSUM
    together with the remaining product i-slices and the self-loop matmul
    (x^T slices against the loop weight).
"""
import sys
if '/opt/trn_rl_repo' not in sys.path:
    sys.path.insert(0, '/opt/trn_rl_repo')

import numpy as np
import ml_dtypes

import concourse.bass as bass
import concourse.bacc as bacc
import concourse.mybir as mybir
import concourse.tile as tile
from concourse.bass_utils import run_bass_kernel_spmd

# problem constants (hardcoded per spec)
NN = 50000      # num nodes
H = 500         # hidden dim
NB = 100        # num bases
SUB = 5         # block size
W_COLS = NB * SUB * SUB  # 2500
NR2 = 474       # num relations * 2
E = 100000      # num edges
NDEV = 8
P = 128
NPD = NN // NDEV          # 6250 nodes per device
NCH = (NPD + P - 1) // P  # 49 chunks
N_PAD = NCH * P           # 6272
KQ4 = 512  # K padded to 4*128 (zero rows beyond 500)
NADD = 3   # i-slice pairwise adds done on DVE (0..4); PE does 5-NADD matmuls

BF = mybir.dt.bfloat16
F32 = mybir.dt.float32
I32 = mybir.dt.int32

_cache = {}


def _plan(src, dst, etype, norm):
    """Host-side sharding plan; layer-invariant."""
    src = np.asarray(src).astype(np.int64)
    dst = np.asarray(dst).astype(np.int64)
    etype = np.asarray(etype).astype(np.int64)
    norm = np.asarray(norm).astype(np.float32).reshape(-1)

    dev_of = dst // NPD
    per = []
    for d in range(NDEV):
        sel = np.nonzero(dev_of == d)[0]
        dl = dst[sel] - d * NPD
        order = np.argsort(dl, kind='stable')
        el = sel[order]
        per.append((el, dl[order]))
    n_max = max(len(el) for el, _ in per)
    ET = (n_max + P - 1) // P

    # unique-src compaction (reduces per-device upload of the gather table)
    uniqs, srcn_cols = [], []
    for el, _ in per:
        u, inv = np.unique(src[el], return_inverse=True)
        uniqs.append(u)
        srcn_cols.append(inv)
    U_PAD = max(len(u) for u in uniqs)

    # per-chunk union windows over edge tiles (same for all devices)
    W0 = np.zeros(NCH, np.int64)
    WEND = np.zeros(NCH, np.int64)
    for c in range(NCH):
        lo, hi = [], []
        for el, dl in per:
            e0 = np.searchsorted(dl, c * P, 'left')
            e1 = np.searchsorted(dl, (c + 1) * P, 'left')
            lo.append(e0 // P)
            hi.append((e1 + P - 1) // P if e1 > 0 else 0)
        W0[c] = min(lo)
        WEND[c] = max(max(hi), W0[c] + 1)
    WEND = np.minimum(WEND, ET)
    W0 = np.minimum(W0, WEND - 1)
    KE = (WEND - W0).astype(np.int64)
    OHT = int(KE.sum())           # total one-hot tiles
    ohoff = np.concatenate([[0], np.cumsum(KE)])[:NCH].astype(np.int64)

    # per-device static input arrays
    srcn = np.zeros((NDEV, P, ET), np.int32)
    etn = np.zeros((NDEV, P, ET), np.int32)
    oh = np.zeros((NDEV, OHT * P, P), np.float32)
    for d in range(NDEV):
        el, dl = per[d]
        n_d = len(el)
        pad = ET * P - n_d
        col = np.pad(srcn_cols[d], (0, pad)).astype(np.int32)
        srcn[d] = col.reshape(ET, P).T
        etn[d] = np.pad(etype[el], (0, pad)).astype(np.int32).reshape(ET, P).T
        nr = norm[el]
        for c in range(NCH):
            for kk in range(KE[c]):
                g0 = (W0[c] + kk) * P
                rows = np.arange(g0, g0 + P)
                valid = rows < n_d
                m = dl[rows[valid]] - c * P
                ok = (m >= 0) & (m < P)
                j = np.nonzero(valid)[0][ok]
                oh[d, (ohoff[c] + kk) * P + j, m[ok]] = nr[rows[valid]][ok]

    return dict(ET=ET, U_PAD=U_PAD, uniqs=uniqs, srcn=srcn, etn=etn,
                oh=oh.astype(ml_dtypes.bfloat16), W0=W0, KE=KE, ohoff=ohoff,
                OHT=OHT)


def _build_nc(ET, U_PAD, W0, KE, ohoff, OHT):
    nc = bacc.Bacc(None, target_bir_lowering=False)

    xg = nc.dram_tensor("xg", [U_PAD, H], BF, kind="ExternalInput")
    xtp = nc.dram_tensor("xtp", [P, 4, N_PAD], BF, kind="ExternalInput")
    wf = nc.dram_tensor("wf", [NR2, W_COLS], BF, kind="ExternalInput")
    lw = nc.dram_tensor("lw", [KQ4, H], BF, kind="ExternalInput")
    srcn = nc.dram_tensor("srcn", [P, ET], I32, kind="ExternalInput")
    etn = nc.dram_tensor("etn", [P, ET], I32, kind="ExternalInput")
    oh = nc.dram_tensor("oh", [OHT * P, P], BF, kind="ExternalInput")
    out = nc.dram_tensor("out", [N_PAD, H], F32, kind="ExternalOutput")

    NMM = SUB - NADD  # product slices fed to PE per window tile

    with tile.TileContext(nc) as tc:
        with tc.tile_pool(name="const", bufs=1) as constp, \
             tc.tile_pool(name="s1", bufs=3) as s1, \
             tc.tile_pool(name="prodp", bufs=10) as prodp, \
             tc.tile_pool(name="s2", bufs=4) as s2, \
             tc.tile_pool(name="psum", bufs=4, space="PSUM") as psp:

            # preload loop weights (rhs tiles, K on partitions) and indices
            lw_sb = []
            for q in range(4):
                t = constp.tile([P, H], BF, tag=f"lw{q}")
                nc.sync.dma_start(out=t[:], in_=lw[q * 128:(q + 1) * 128, :])
                lw_sb.append(t)
            srcn_sb = constp.tile([P, ET], I32, tag="srcn")
            etn_sb = constp.tile([P, ET], I32, tag="etn")
            nc.sync.dma_start(out=srcn_sb[:], in_=srcn[:, :])
            nc.sync.dma_start(out=etn_sb[:], in_=etn[:, :])

            prods = {}   # edge-tile idx -> list of NMM rhs views (+ backing tiles)

            def produce(t):
                xe = s1.tile([P, H], BF, tag="xe")
                wg = s1.tile([P, W_COLS], BF, tag="wg")
                nc.gpsimd.indirect_dma_start(
                    out=xe[:], out_offset=None, in_=xg[:, :],
                    in_offset=bass.IndirectOffsetOnAxis(ap=srcn_sb[:, t:t + 1], axis=0))
                nc.gpsimd.indirect_dma_start(
                    out=wg[:], out_offset=None, in_=wf[:, :],
                    in_offset=bass.IndirectOffsetOnAxis(ap=etn_sb[:, t:t + 1], axis=0))
                # expand xe[b*5+i] to [i, b, j] layout (broadcast over j) on ACT
                xex = s1.tile([P, W_COLS], BF, tag="xex")
                xe_v = xe[:].rearrange("p (b i) -> p i b", i=SUB)  # strided view
                nc.scalar.activation(
                    out=xex[:].rearrange("p (i b j) -> p i b j", i=SUB, j=SUB),
                    in_=xe_v.to_broadcast([P, SUB, NB, SUB]),
                    func=mybir.ActivationFunctionType.Copy)
                # one full-width multiply: all 2500 partial products
                prod = prodp.tile([P, W_COLS], BF, tag="prod")
                nc.vector.tensor_tensor(out=prod[:], in0=xex[:], in1=wg[:],
                                        op=mybir.AluOpType.mult)
                # fold NADD i-slices pairwise on DVE
                sl = [prod[:, i * H:(i + 1) * H] for i in range(SUB)]
                if NADD >= 1:
                    s01 = prodp.tile([P, H], BF, tag="s01")
                    nc.vector.tensor_tensor(out=s01[:], in0=sl[0], in1=sl[1],
                                            op=mybir.AluOpType.add)
                    sl = [s01[:]] + sl[2:]
                if NADD >= 2:
                    s23 = prodp.tile([P, H], BF, tag="s23")
                    nc.vector.tensor_tensor(out=s23[:], in0=sl[1], in1=sl[2],
                                            op=mybir.AluOpType.add)
                    sl = [sl[0], s23[:]] + sl[3:]
                if NADD >= 3:
                    s03 = prodp.tile([P, H], BF, tag="s03")
                    nc.vector.tensor_tensor(out=s03[:], in0=sl[0], in1=sl[1],
                                            op=mybir.AluOpType.add)
                    sl = [s03[:]] + sl[2:]
                if NADD >= 4:
                    s04 = prodp.tile([P, H], BF, tag="s04")
                    nc.vector.tensor_tensor(out=s04[:], in0=sl[0], in1=sl[1],
                                            op=mybir.AluOpType.add)
                    sl = [s04[:]] + sl[2:]
                assert len(sl) == NMM
                prods[t] = sl

            produced = 0
            for c in range(NCH):
                need = int(W0[c] + KE[c])
                while produced < need:
                    produce(produced)
                    produced += 1
                ps = psp.tile([P, H], F32, tag="ps")
                ke = int(KE[c])
                ohsb = s2.tile([P, 7 * P], BF, tag="ohsb")
                o0 = int(ohoff[c]) * P
                nc.sync.dma_start(
                    out=ohsb[:, :ke * P].rearrange("p (k m) -> p k m", k=ke),
                    in_=oh[o0:o0 + ke * P, :].rearrange("(k p) m -> p k m", p=P))
                xt = s2.tile([P, 4, P], BF, tag="xt")
                nc.sync.dma_start(out=xt[:], in_=xtp[:, :, c * P:(c + 1) * P])
                first = True
                for kk in range(ke):
                    t = int(W0[c]) + kk
                    for rv in prods[t]:
                        nc.tensor.matmul(out=ps[:],
                                         lhsT=ohsb[:, kk * P:(kk + 1) * P],
                                         rhs=rv, start=first, stop=False)
                        first = False
                for q in range(4):
                    nc.tensor.matmul(out=ps[:], lhsT=xt[:, q, :],
                                     rhs=lw_sb[q][:],
                                     start=False, stop=(q == 3))
                outt = s2.tile([P, H], F32, tag="outt")
                nc.scalar.activation(out=outt[:], in_=ps[:],
                                     func=mybir.ActivationFunctionType.Copy)
                nc.sync.dma_start(out=out[c * P:(c + 1) * P, :], in_=outt[:])
                # drop window tiles no longer needed
                if c + 1 < NCH:
                    for t in [k for k in prods if k < int(W0[c + 1])]:
                        del prods[t]
    nc.finalize()
    return nc


def _run_layer(nc, plan, x, wfp, lwb, trace=False):
    """One RGCN-BDD layer (pre-bias, pre-activation) on 8 cores."""
    xb = x.astype(ml_dtypes.bfloat16)
    in_maps = []
    for d in range(NDEV):
        u = plan['uniqs'][d]
        xgd = np.zeros((plan['U_PAD'], H), ml_dtypes.bfloat16)
        xgd[:len(u)] = xb[u]
        xtpd = np.zeros((P, 4, N_PAD), ml_dtypes.bfloat16)
        xs = xb[d * NPD:(d + 1) * NPD].T  # [500, NPD]
        for q in range(4):
            rows = min(128, H - q * 128)
            xtpd[:rows, q, :NPD] = xs[q * 128:q * 128 + rows]
        in_maps.append({
            "xg": xgd, "xtp": np.ascontiguousarray(xtpd), "wf": wfp, "lw": lwb,
            "srcn": plan['srcn'][d], "etn": plan['etn'][d], "oh": plan['oh'][d],
        })
    res = run_bass_kernel_spmd(nc, in_maps, core_ids=list(range(NDEV)),
                               trace=trace)
    outp = np.empty((NN, H), np.float32)
    for d in range(NDEV):
        outp[d * NPD:(d + 1) * NPD] = res.results[d]["out"][:NPD]
    return outp, res


def _pad_lw(lw):
    lwp = np.zeros((KQ4, H), np.float32)
    lwp[:H] = np.asarray(lw, np.float32)
    return lwp.astype(ml_dtypes.bfloat16)


def _permute_w(W):
    # [r, b, i, j] -> [r, i, b, j] flattened, bf16
    W = np.asarray(W, dtype=np.float32).reshape(NR2, NB, SUB, SUB)
    return np.ascontiguousarray(
        W.transpose(0, 2, 1, 3).reshape(NR2, W_COLS)).astype(ml_dtypes.bfloat16)


def kernel(nids, src, dst, etype, norm, emb, W1, loop_w1, bias1,
           W2, loop_w2, bias2, _trace=False, _times=None):
    key = "nc"
    if key not in _cache:
        plan = _plan(src, dst, etype, norm)
        nc = _build_nc(plan['ET'], plan['U_PAD'], plan['W0'], plan['KE'],
                       plan['ohoff'], plan['OHT'])
        _cache[key] = (plan, nc)
    plan, nc = _cache[key]

    x = np.asarray(emb, dtype=np.float32)[np.asarray(nids, dtype=np.int64)]
    h_pre, r1 = _run_layer(nc, plan, x, _permute_w(W1), _pad_lw(loop_w1),
                           trace=_trace)
    h = np.maximum(h_pre + np.asarray(bias1, dtype=np.float32)[None, :], 0.0)
    out_pre, r2 = _run_layer(nc, plan, h, _permute_w(W2), _pad_lw(loop_w2),
                             trace=_trace)
    out = out_pre + np.asarray(bias2, dtype=np.float32)[None, :]
    if _times is not None:
        _times.extend([r1, r2])
    return out
